# revision 8
# baseline (speedup 1.0000x reference)
"""Trainium2 Bass kernel: unnormalized single-head attention block.

Computes, for x [4, 4096, 1024] and w_q/w_k/w_v/w_o [1024, 1024] (all fp32):
    q = x @ w_q ; k = x @ w_k ; v = x @ w_v
    scores = q @ k.T            (no softmax)
    out = (scores @ v) @ w_o

Because there is no softmax, the chain is associative and collapses to
    out_b = x_b @ [ w_q @ w_k.T @ (x_b.T @ x_b) @ w_v @ w_o ]
which replaces the two T x T matmuls (34 GFLOP each per batch) with a
Gram matrix G_b = x_b.T @ x_b and a short chain of 1024^3 matmuls:
~90 GFLOP total instead of ~412 GFLOP.

Sharding: 8 NeuronCores = (4 batches) x (2 sequence halves). Each core
computes G over its own 2048-row half; the pair's halves are summed with a
2 MB bf16 AllReduce over groups [[0,1],[2,3],[4,5],[6,7]] (G_b = own + peer).
While the collective is in flight the PE computes the batch-independent
products AT = w_k @ w_q.T and C = w_v @ w_o, so the tensor engine never
idles. Afterwards R = G @ C (G is symmetric, so G serves as its own lhsT),
M = AT.T @ R, and out rows for the own half: out = x_own @ M.

Device math is bf16 with fp32 PSUM accumulation. The host ships bf16
tensors directly (x half in both natural and transposed layout; w_q/w_k/w_v
transposed) so no on-device transposes or casts are needed.
"""

import contextlib
import ctypes
import os
import sys
import types

import numpy as np

B = 4
T = 4096
D = 1024
H = T // 2          # rows per core
P = 128             # SBUF partitions
NCORES = 8
DT = D // P         # 8 tiles along any 1024 dim
TT = H // P         # 16 own-half t-tiles
FREE = 512          # matmul moving free dim / PSUM bank width (fp32)
KC = D // FREE      # 2 free-dim chunks of 512 along a 1024 dim
GROUPS = [[0, 1], [2, 3], [4, 5], [6, 7]]

_STATE = {}
LAST_RESULTS = None


def _install_axon_ntff_shim():
    """bass_utils(trace=True) under axon imports antenv.axon_hooks, which the
    agent image lacks. Provide the documented ctypes equivalent so tracing
    works; degrades to hook=None when the .so has no profile symbols."""
    try:
        import antenv.axon_hooks  # noqa: F401
        return
    except ImportError:
        pass

    so_path = "/opt/axon/libaxon_pjrt.so"

    def _make_hook():
        try:
            lib = ctypes.CDLL(so_path)
        except OSError:
            return None
        if not hasattr(lib, "axon_start_nrt_profile"):
            return None
        lib.axon_start_nrt_profile.argtypes = [
            ctypes.POINTER(ctypes.c_int64),
            ctypes.c_size_t,
        ]
        lib.axon_start_nrt_profile.restype = ctypes.c_int64
        lib.axon_stop_nrt_profile.argtypes = [ctypes.c_char_p]
        lib.axon_stop_nrt_profile.restype = ctypes.c_int64

        @contextlib.contextmanager
        def _hook(output_dir, device_ids):
            import jax

            jax.devices()
            if device_ids:
                ids = (ctypes.c_int64 * len(device_ids))(*device_ids)
                rc = lib.axon_start_nrt_profile(ids, len(device_ids))
            else:
                rc = lib.axon_start_nrt_profile(None, 0)
            if rc != 0:
                raise RuntimeError(f"axon_start_nrt_profile rc={rc}")
            try:
                yield
            finally:
                n = lib.axon_stop_nrt_profile(str(output_dir).encode())
                print(f"profile: {n} file(s) written to {output_dir}", file=sys.stderr)

        return _hook

    mod = types.ModuleType("antenv.axon_hooks")
    mod.get_axon_ntff_profile_hook = _make_hook
    mod.set_axon_ntff_profile_hook = lambda h: None
    sys.modules["antenv.axon_hooks"] = mod


def _trace_kernel(tc, xn, xt, wqT, wkT, wvT, wo, out):
    import concourse.mybir as mybir
    from concourse.bass import ts

    nc = tc.nc
    f32 = mybir.dt.float32
    bf16 = mybir.dt.bfloat16

    with contextlib.ExitStack() as top:
        ps_pool = top.enter_context(tc.tile_pool(name="ps", bufs=8, space="PSUM"))
        dram_pool = top.enter_context(tc.tile_pool(name="cdram", bufs=2, space="DRAM"))
        at_pool = top.enter_context(tc.tile_pool(name="at", bufs=DT))
        c_pool = top.enter_context(tc.tile_pool(name="c", bufs=DT))

        # Collective staging in local DRAM (pair groups need Local addr space).
        # The pairwise G AllReduce is split into two 1 MB chunks so the first
        # half of G is in flight while the second half is still computing.
        HB = DT // 2
        gsrc = [
            dram_pool.tile([HB, P, D], bf16, name=f"gsrc{h}", tag="gsrc")
            for h in range(2)
        ]
        gsum = [
            dram_pool.tile([HB, P, D], bf16, name=f"gsum{h}", tag="gsum")
            for h in range(2)
        ]

        with contextlib.ExitStack() as setup:
            xn_pool = setup.enter_context(tc.tile_pool(name="xn", bufs=TT))
            w_pool = setup.enter_context(tc.tile_pool(name="w", bufs=4 * DT))
            gown_pool = setup.enter_context(tc.tile_pool(name="gown", bufs=DT))

            xns = []
            for t in range(TT):
                xv = xn_pool.tile([P, D], bf16, name=f"xn{t}", tag="xn")
                nc.sync.dma_start(out=xv[:], in_=xn[ts(t, P), :])
                xns.append(xv)

            def load_w(w_ap, tag):
                tiles = []
                for i in range(DT):
                    wt = w_pool.tile([P, D], bf16, name=f"{tag}{i}", tag="w")
                    nc.sync.dma_start(out=wt[:], in_=w_ap[ts(i, P), :])
                    tiles.append(wt)
                return tiles

            wk_t = load_w(wkT, "wk")
            wq_t = load_w(wqT, "wq")
            wv_t = load_w(wvT, "wv")
            wo_t = load_w(wo, "wo")

            # --- own-half Gram matrix G[j,k] = sum_t x[t,j] x[t,k] ---
            gown = [
                gown_pool.tile([P, D], bf16, name=f"go{j}", tag="gown")
                for j in range(DT)
            ]
            for jt in range(DT):
                for kc in range(KC):
                    psum = ps_pool.tile([P, FREE], f32, name="psg", tag="ps")
                    for t in range(TT):
                        nc.tensor.matmul(
                            psum[:],
                            xns[t][:, ts(jt, P)],
                            xns[t][:, ts(kc, FREE)],
                            start=(t == 0),
                            stop=(t == TT - 1),
                        )
                    nc.vector.tensor_copy(gown[jt][:, ts(kc, FREE)], psum[:])
                nc.scalar.dma_start(out=gsrc[jt // HB][jt % HB], in_=gown[jt][:])
                if jt % HB == HB - 1:
                    # Pairwise sum of this chunk: G_own + G_peer = full G rows.
                    nc.gpsimd.collective_compute(
                        "AllReduce",
                        mybir.AluOpType.add,
                        replica_groups=GROUPS,
                        ins=[gsrc[jt // HB].opt()],
                        outs=[gsum[jt // HB].opt()],
                    )

            # --- batch-independent products, overlapped with the collective ---
            # AT[j,d] = (w_q @ w_k.T).T = sum_i wk[j,i] wq[d,i]
            ats = [
                at_pool.tile([P, D], bf16, name=f"at{j}", tag="at") for j in range(DT)
            ]
            for jt in range(DT):
                for dc in range(KC):
                    psum = ps_pool.tile([P, FREE], f32, name="psa", tag="ps")
                    for i in range(DT):
                        nc.tensor.matmul(
                            psum[:],
                            wk_t[i][:, ts(jt, P)],
                            wq_t[i][:, ts(dc, FREE)],
                            start=(i == 0),
                            stop=(i == DT - 1),
                        )
                    nc.vector.tensor_copy(ats[jt][:, ts(dc, FREE)], psum[:])

            # C[k,e] = (w_v @ w_o)[k,e] = sum_l wv[k,l] wo[l,e]
            cs = [c_pool.tile([P, D], bf16, name=f"c{k}", tag="c") for k in range(DT)]
            for kt in range(DT):
                for ec in range(KC):
                    psum = ps_pool.tile([P, FREE], f32, name="psc", tag="ps")
                    for l in range(DT):
                        nc.tensor.matmul(
                            psum[:],
                            wv_t[l][:, ts(kt, P)],
                            wo_t[l][:, ts(ec, FREE)],
                            start=(l == 0),
                            stop=(l == DT - 1),
                        )
                    nc.vector.tensor_copy(cs[kt][:, ts(ec, FREE)], psum[:])

        # Late-phase pools, created after the setup pools release their SBUF.
        xt_pool = top.enter_context(tc.tile_pool(name="xt", bufs=DT))
        gf_pool = top.enter_context(tc.tile_pool(name="gf", bufs=DT))
        r_pool = top.enter_context(tc.tile_pool(name="r", bufs=DT))
        m_pool = top.enter_context(tc.tile_pool(name="m", bufs=DT))
        ot_pool = top.enter_context(tc.tile_pool(name="ot", bufs=4))

        # x.T tiles for the final out = x @ M matmul.
        xts = []
        for i in range(DT):
            xv = xt_pool.tile([P, H], bf16, name=f"xt{i}", tag="xt")
            nc.sync.dma_start(out=xv[:], in_=xt[ts(i, P), :])
            xts.append(xv)

        # Full G into SBUF (waits on the AllReduce via tile deps; rides the
        # otherwise-idle SWDGE queue so the wait cannot stall the load queues).
        gfs = []
        for kt in range(DT):
            gf = gf_pool.tile([P, D], bf16, name=f"gf{kt}", tag="gf")
            nc.gpsimd.dma_start(out=gf[:], in_=gsum[kt // HB][kt % HB])
            gfs.append(gf)

        # R[j,e] = (G @ C)[j,e]; G is symmetric so its tiles serve as lhsT.
        rs = []
        for jt in range(DT):
            rt = r_pool.tile([P, D], bf16, name=f"r{jt}", tag="r")
            for ec in range(KC):
                psum = ps_pool.tile([P, FREE], f32, name="psr", tag="ps")
                for kt in range(DT):
                    nc.tensor.matmul(
                        psum[:],
                        gfs[kt][:, ts(jt, P)],
                        cs[kt][:, ts(ec, FREE)],
                        start=(kt == 0),
                        stop=(kt == DT - 1),
                    )
                nc.vector.tensor_copy(rt[:, ts(ec, FREE)], psum[:])
            rs.append(rt)

        # M[d,e] = (w_q @ w_k.T @ R)[d,e] = sum_j AT[j,d] R[j,e]
        ms = []
        for dt_ in range(DT):
            mt = m_pool.tile([P, D], bf16, name=f"m{dt_}", tag="m")
            for ec in range(KC):
                psum = ps_pool.tile([P, FREE], f32, name="psm", tag="ps")
                for jt in range(DT):
                    nc.tensor.matmul(
                        psum[:],
                        ats[jt][:, ts(dt_, P)],
                        rs[jt][:, ts(ec, FREE)],
                        start=(jt == 0),
                        stop=(jt == DT - 1),
                    )
                nc.vector.tensor_copy(mt[:, ts(ec, FREE)], psum[:])
            ms.append(mt)

        # out[t,e] = sum_d x[t,d] M[d,e], own-half rows.
        for tt in range(TT):
            for ec in range(KC):
                psum = ps_pool.tile([P, FREE], f32, name="pso", tag="ps")
                for dt_ in range(DT):
                    nc.tensor.matmul(
                        psum[:],
                        xts[dt_][:, ts(tt, P)],
                        ms[dt_][:, ts(ec, FREE)],
                        start=(dt_ == 0),
                        stop=(dt_ == DT - 1),
                    )
                o = ot_pool.tile([P, FREE], f32, name="ot", tag="ot")
                nc.scalar.copy(o[:], psum[:])
                nc.scalar.dma_start(out=out[ts(tt, P), ts(ec, FREE)], in_=o[:])


def _build():
    _install_axon_ntff_shim()
    import concourse.mybir as mybir
    import concourse.tile as tile
    from concourse import bacc

    f32 = mybir.dt.float32
    bf16 = mybir.dt.bfloat16
    nc = bacc.Bacc("TRN2", target_bir_lowering=False, debug=False, num_devices=NCORES)
    xn = nc.dram_tensor("xn", [H, D], bf16, kind="ExternalInput").ap()
    xt = nc.dram_tensor("xt", [D, H], bf16, kind="ExternalInput").ap()
    wqT = nc.dram_tensor("wqT", [D, D], bf16, kind="ExternalInput").ap()
    wkT = nc.dram_tensor("wkT", [D, D], bf16, kind="ExternalInput").ap()
    wvT = nc.dram_tensor("wvT", [D, D], bf16, kind="ExternalInput").ap()
    wo = nc.dram_tensor("wo", [D, D], bf16, kind="ExternalInput").ap()
    out = nc.dram_tensor("out", [H, D], f32, kind="ExternalOutput").ap()

    with tile.TileContext(nc) as tc:
        _trace_kernel(tc, xn, xt, wqT, wkT, wvT, wo, out)
    nc.compile()
    return nc


def kernel(x, w_q, w_k, w_v, w_o):
    global LAST_RESULTS
    import ml_dtypes
    from concourse import bass_utils

    if "nc" not in _STATE:
        _STATE["nc"] = _build()
    nc = _STATE["nc"]

    bf16 = ml_dtypes.bfloat16
    x = np.ascontiguousarray(x, dtype=np.float32)
    wqT = np.asarray(w_q, dtype=np.float32).T.astype(bf16)
    wkT = np.asarray(w_k, dtype=np.float32).T.astype(bf16)
    wvT = np.asarray(w_v, dtype=np.float32).T.astype(bf16)
    wob = np.ascontiguousarray(np.asarray(w_o, dtype=np.float32)).astype(bf16)

    in_maps = []
    for core in range(NCORES):
        b, half = core // 2, core % 2
        xh = x[b, half * H : (half + 1) * H]
        in_maps.append(
            {
                "xn": xh.astype(bf16),
                "xt": xh.T.astype(bf16),
                "wqT": wqT,
                "wkT": wkT,
                "wvT": wvT,
                "wo": wob,
            }
        )

    LAST_RESULTS = bass_utils.run_bass_kernel_spmd(
        nc, in_maps, core_ids=list(range(NCORES))
    )
    out = np.empty((B, T, D), dtype=np.float32)
    for core in range(NCORES):
        b, half = core // 2, core % 2
        out[b, half * H : (half + 1) * H] = LAST_RESULTS.results[core]["out"]
    return out


# revision 13
# speedup vs baseline: 1.1979x; 1.1979x over previous
"""Trainium2 Bass kernel: unnormalized single-head attention block.

Computes, for x [4, 4096, 1024] and w_q/w_k/w_v/w_o [1024, 1024] (all fp32):
    q = x @ w_q ; k = x @ w_k ; v = x @ w_v
    scores = q @ k.T            (no softmax)
    out = (scores @ v) @ w_o

Because there is no softmax, the chain is associative and collapses to
    out_b = x_b @ [ w_q @ w_k.T @ (x_b.T @ x_b) @ w_v @ w_o ]
which replaces the two T x T matmuls (34 GFLOP each per batch) with a
Gram matrix G_b = x_b.T @ x_b and a short chain of 1024^3 matmuls:
~90 GFLOP total instead of ~412 GFLOP.

Sharding: 8 NeuronCores = (4 batches) x (2 sequence halves). Each core
computes G over its own 2048-row half; the pair's halves are summed with a
2 MB bf16 AllReduce over groups [[0,1],[2,3],[4,5],[6,7]] (G_b = own + peer).
While the collective is in flight the PE computes the batch-independent
products AT = w_k @ w_q.T and C = w_v @ w_o, so the tensor engine never
idles. Afterwards R = G @ C (G is symmetric, so G serves as its own lhsT),
M = AT.T @ R, and out rows for the own half: out = x_own @ M.

Device math is bf16 with fp32 PSUM accumulation. The host ships bf16
tensors directly (x half in both natural and transposed layout; w_q/w_k/w_v
transposed) so no on-device transposes or casts are needed.
"""

import contextlib
import ctypes
import os
import sys
import types

import numpy as np

B = 4
T = 4096
D = 1024
H = T // 2          # rows per core
P = 128             # SBUF partitions
NCORES = 8
DT = D // P         # 8 tiles along any 1024 dim
TT = H // P         # 16 own-half t-tiles
FREE = 512          # matmul moving free dim / PSUM bank width (fp32)
KC = D // FREE      # 2 free-dim chunks of 512 along a 1024 dim
GROUPS = [[0, 1], [2, 3], [4, 5], [6, 7]]
NCHUNK = int(os.environ.get("K_NCHUNK", "1"))  # G-AllReduce chunk count
CCKIND = os.environ.get("K_CCKIND", "AR")  # AR=AllReduce, AG=AllGather+local add
WARMUP = int(os.environ.get("K_WARMUP", "0"))  # dummy matmuls to warm the PE

_STATE = {}
LAST_RESULTS = None


def _install_axon_ntff_shim():
    """bass_utils(trace=True) under axon imports antenv.axon_hooks, which the
    agent image lacks. Provide the documented ctypes equivalent so tracing
    works; degrades to hook=None when the .so has no profile symbols."""
    try:
        import antenv.axon_hooks  # noqa: F401
        return
    except ImportError:
        pass

    so_path = "/opt/axon/libaxon_pjrt.so"

    def _make_hook():
        try:
            lib = ctypes.CDLL(so_path)
        except OSError:
            return None
        if not hasattr(lib, "axon_start_nrt_profile"):
            return None
        lib.axon_start_nrt_profile.argtypes = [
            ctypes.POINTER(ctypes.c_int64),
            ctypes.c_size_t,
        ]
        lib.axon_start_nrt_profile.restype = ctypes.c_int64
        lib.axon_stop_nrt_profile.argtypes = [ctypes.c_char_p]
        lib.axon_stop_nrt_profile.restype = ctypes.c_int64

        @contextlib.contextmanager
        def _hook(output_dir, device_ids):
            import jax

            jax.devices()
            if device_ids:
                ids = (ctypes.c_int64 * len(device_ids))(*device_ids)
                rc = lib.axon_start_nrt_profile(ids, len(device_ids))
            else:
                rc = lib.axon_start_nrt_profile(None, 0)
            if rc != 0:
                raise RuntimeError(f"axon_start_nrt_profile rc={rc}")
            try:
                yield
            finally:
                n = lib.axon_stop_nrt_profile(str(output_dir).encode())
                print(f"profile: {n} file(s) written to {output_dir}", file=sys.stderr)

        return _hook

    mod = types.ModuleType("antenv.axon_hooks")
    mod.get_axon_ntff_profile_hook = _make_hook
    mod.set_axon_ntff_profile_hook = lambda h: None
    sys.modules["antenv.axon_hooks"] = mod


def _trace_kernel(tc, xn, xt, wqT, wkT, wvT, wo, out):
    import concourse.mybir as mybir
    from concourse.bass import ts

    nc = tc.nc
    f32 = mybir.dt.float32
    bf16 = mybir.dt.bfloat16

    with contextlib.ExitStack() as top:
        ps_pool = top.enter_context(tc.tile_pool(name="ps", bufs=8, space="PSUM"))
        dram_pool = top.enter_context(tc.tile_pool(name="cdram", bufs=2, space="DRAM"))
        at_pool = top.enter_context(tc.tile_pool(name="at", bufs=DT))
        c_pool = top.enter_context(tc.tile_pool(name="c", bufs=DT))

        # Collective staging in local DRAM (pair groups need Local addr space).
        # The pairwise G AllReduce can be split into chunks so early G rows
        # are in flight while later ones are still computing.
        HB = DT // NCHUNK
        gsrc = [
            dram_pool.tile([HB, P, D], bf16, name=f"gsrc{h}", tag="gsrc")
            for h in range(NCHUNK)
        ]
        gsum = [
            dram_pool.tile([HB, P, D], bf16, name=f"gsum{h}", tag="gsum")
            for h in range(NCHUNK)
        ]

        with contextlib.ExitStack() as setup:
            xn_pool = setup.enter_context(tc.tile_pool(name="xn", bufs=TT))
            w_pool = setup.enter_context(tc.tile_pool(name="w", bufs=4 * DT))
            gown_pool = setup.enter_context(tc.tile_pool(name="gown", bufs=DT))

            xns = []
            for t in range(TT):
                xv = xn_pool.tile([P, D], bf16, name=f"xn{t}", tag="xn")
                nc.sync.dma_start(out=xv[:], in_=xn[ts(t, P), :])
                xns.append(xv)

            def load_w(w_ap, tag):
                tiles = []
                for i in range(DT):
                    wt = w_pool.tile([P, D], bf16, name=f"{tag}{i}", tag="w")
                    nc.sync.dma_start(out=wt[:], in_=w_ap[ts(i, P), :])
                    tiles.append(wt)
                return tiles

            wk_t = load_w(wkT, "wk")
            wq_t = load_w(wqT, "wq")
            wv_t = load_w(wvT, "wv")
            wo_t = load_w(wo, "wo")

            # --- own-half Gram matrix G[j,k] = sum_t x[t,j] x[t,k] ---
            gown = [
                gown_pool.tile([P, D], bf16, name=f"go{j}", tag="gown")
                for j in range(DT)
            ]
            for jt in range(DT):
                for kc in range(KC):
                    psum = ps_pool.tile([P, FREE], f32, name="psg", tag="ps")
                    for t in range(TT):
                        nc.tensor.matmul(
                            psum[:],
                            xns[t][:, ts(jt, P)],
                            xns[t][:, ts(kc, FREE)],
                            start=(t == 0),
                            stop=(t == TT - 1),
                        )
                    nc.vector.tensor_copy(gown[jt][:, ts(kc, FREE)], psum[:])
                nc.scalar.dma_start(out=gsrc[jt // HB][jt % HB], in_=gown[jt][:])
                if jt % HB == HB - 1:
                    # Pairwise sum of this chunk: G_own + G_peer = full G rows.
                    nc.gpsimd.collective_compute(
                        "AllReduce",
                        mybir.AluOpType.add,
                        replica_groups=GROUPS,
                        ins=[gsrc[jt // HB].opt()],
                        outs=[gsum[jt // HB].opt()],
                    )

            # --- batch-independent products, overlapped with the collective ---
            # AT[j,d] = (w_q @ w_k.T).T = sum_i wk[j,i] wq[d,i]
            ats = [
                at_pool.tile([P, D], bf16, name=f"at{j}", tag="at") for j in range(DT)
            ]
            for jt in range(DT):
                for dc in range(KC):
                    psum = ps_pool.tile([P, FREE], f32, name="psa", tag="ps")
                    for i in range(DT):
                        nc.tensor.matmul(
                            psum[:],
                            wk_t[i][:, ts(jt, P)],
                            wq_t[i][:, ts(dc, FREE)],
                            start=(i == 0),
                            stop=(i == DT - 1),
                        )
                    nc.vector.tensor_copy(ats[jt][:, ts(dc, FREE)], psum[:])

            # C[k,e] = (w_v @ w_o)[k,e] = sum_l wv[k,l] wo[l,e]
            cs = [c_pool.tile([P, D], bf16, name=f"c{k}", tag="c") for k in range(DT)]
            for kt in range(DT):
                for ec in range(KC):
                    psum = ps_pool.tile([P, FREE], f32, name="psc", tag="ps")
                    for l in range(DT):
                        nc.tensor.matmul(
                            psum[:],
                            wv_t[l][:, ts(kt, P)],
                            wo_t[l][:, ts(ec, FREE)],
                            start=(l == 0),
                            stop=(l == DT - 1),
                        )
                    nc.vector.tensor_copy(cs[kt][:, ts(ec, FREE)], psum[:])

        # Late-phase pools, created after the setup pools release their SBUF.
        xt_pool = top.enter_context(tc.tile_pool(name="xt", bufs=DT))
        gf_pool = top.enter_context(tc.tile_pool(name="gf", bufs=DT))
        r_pool = top.enter_context(tc.tile_pool(name="r", bufs=DT))
        m_pool = top.enter_context(tc.tile_pool(name="m", bufs=DT))
        ot_pool = top.enter_context(tc.tile_pool(name="ot", bufs=4))

        # x.T tiles for the final out = x @ M matmul.
        xts = []
        for i in range(DT):
            xv = xt_pool.tile([P, H], bf16, name=f"xt{i}", tag="xt")
            nc.sync.dma_start(out=xv[:], in_=xt[ts(i, P), :])
            xts.append(xv)

        # Full G into SBUF (waits on the AllReduce via tile deps; rides the
        # otherwise-idle SWDGE queue so the wait cannot stall the load queues).
        gfs = []
        for kt in range(DT):
            gf = gf_pool.tile([P, D], bf16, name=f"gf{kt}", tag="gf")
            nc.gpsimd.dma_start(out=gf[:], in_=gsum[kt // HB][kt % HB])
            gfs.append(gf)

        # R[j,e] = (G @ C)[j,e]; G is symmetric so its tiles serve as lhsT.
        rs = []
        for jt in range(DT):
            rt = r_pool.tile([P, D], bf16, name=f"r{jt}", tag="r")
            for ec in range(KC):
                psum = ps_pool.tile([P, FREE], f32, name="psr", tag="ps")
                for kt in range(DT):
                    nc.tensor.matmul(
                        psum[:],
                        gfs[kt][:, ts(jt, P)],
                        cs[kt][:, ts(ec, FREE)],
                        start=(kt == 0),
                        stop=(kt == DT - 1),
                    )
                nc.vector.tensor_copy(rt[:, ts(ec, FREE)], psum[:])
            rs.append(rt)

        # M[d,e] = (w_q @ w_k.T @ R)[d,e] = sum_j AT[j,d] R[j,e]
        ms = []
        for dt_ in range(DT):
            mt = m_pool.tile([P, D], bf16, name=f"m{dt_}", tag="m")
            for ec in range(KC):
                psum = ps_pool.tile([P, FREE], f32, name="psm", tag="ps")
                for jt in range(DT):
                    nc.tensor.matmul(
                        psum[:],
                        ats[jt][:, ts(dt_, P)],
                        rs[jt][:, ts(ec, FREE)],
                        start=(jt == 0),
                        stop=(jt == DT - 1),
                    )
                nc.vector.tensor_copy(mt[:, ts(ec, FREE)], psum[:])
            ms.append(mt)

        # out[t,e] = sum_d x[t,d] M[d,e], own-half rows.
        for tt in range(TT):
            for ec in range(KC):
                psum = ps_pool.tile([P, FREE], f32, name="pso", tag="ps")
                for dt_ in range(DT):
                    nc.tensor.matmul(
                        psum[:],
                        xts[dt_][:, ts(tt, P)],
                        ms[dt_][:, ts(ec, FREE)],
                        start=(dt_ == 0),
                        stop=(dt_ == DT - 1),
                    )
                o = ot_pool.tile([P, FREE], f32, name="ot", tag="ot")
                nc.scalar.copy(o[:], psum[:])
                nc.scalar.dma_start(out=out[ts(tt, P), ts(ec, FREE)], in_=o[:])


def _build():
    _install_axon_ntff_shim()
    import concourse.mybir as mybir
    import concourse.tile as tile
    from concourse import bacc

    f32 = mybir.dt.float32
    bf16 = mybir.dt.bfloat16
    nc = bacc.Bacc("TRN2", target_bir_lowering=False, debug=False, num_devices=NCORES)
    xn = nc.dram_tensor("xn", [H, D], bf16, kind="ExternalInput").ap()
    xt = nc.dram_tensor("xt", [D, H], bf16, kind="ExternalInput").ap()
    wqT = nc.dram_tensor("wqT", [D, D], bf16, kind="ExternalInput").ap()
    wkT = nc.dram_tensor("wkT", [D, D], bf16, kind="ExternalInput").ap()
    wvT = nc.dram_tensor("wvT", [D, D], bf16, kind="ExternalInput").ap()
    wo = nc.dram_tensor("wo", [D, D], bf16, kind="ExternalInput").ap()
    out = nc.dram_tensor("out", [H, D], f32, kind="ExternalOutput").ap()

    with tile.TileContext(nc) as tc:
        _trace_kernel(tc, xn, xt, wqT, wkT, wvT, wo, out)
    nc.compile()
    return nc


def kernel(x, w_q, w_k, w_v, w_o):
    global LAST_RESULTS
    import ml_dtypes
    from concourse import bass_utils

    if "nc" not in _STATE:
        _STATE["nc"] = _build()
    nc = _STATE["nc"]

    bf16 = ml_dtypes.bfloat16
    x = np.ascontiguousarray(x, dtype=np.float32)
    wqT = np.asarray(w_q, dtype=np.float32).T.astype(bf16)
    wkT = np.asarray(w_k, dtype=np.float32).T.astype(bf16)
    wvT = np.asarray(w_v, dtype=np.float32).T.astype(bf16)
    wob = np.ascontiguousarray(np.asarray(w_o, dtype=np.float32)).astype(bf16)

    in_maps = []
    for core in range(NCORES):
        b, half = core // 2, core % 2
        xh = x[b, half * H : (half + 1) * H]
        in_maps.append(
            {
                "xn": xh.astype(bf16),
                "xt": xh.T.astype(bf16),
                "wqT": wqT,
                "wkT": wkT,
                "wvT": wvT,
                "wo": wob,
            }
        )

    LAST_RESULTS = bass_utils.run_bass_kernel_spmd(
        nc, in_maps, core_ids=list(range(NCORES))
    )
    out = np.empty((B, T, D), dtype=np.float32)
    for core in range(NCORES):
        b, half = core // 2, core % 2
        out[b, half * H : (half + 1) * H] = LAST_RESULTS.results[core]["out"]
    return out
